# revision 1
# baseline (speedup 1.0000x reference)
"""LFD all-pairs distance kernel for 8 Trainium2 NeuronCores.

Strategy (data-parallel over tgt batch axis m, per sharding hint):
  - Each of the 8 cores owns 16 of the 128 tgt rows.
  - The pairwise cost D[s, t] = sum_k w_k * q8_table[idxS[s,k], idxT[t,k]]
    (s = 400 src descriptors (n,sc,sa), t = 1600 tgt descriptors (m_loc,tc,ta),
     k = 47 coefficient slots: 35 art + 10 fd(x2) + cir(x2) + ecc)
    is computed on-device as a dense TensorE matmul:
        D[t_tile, s] = B^T @ Rt
    where Rt[(k,c), s] = w_k * q8_table[idxS[s,k], c]   (row-gathered table)
          B[(k,c), t] = 1 if idxT[t,k] == c else 0      (one-hot encoding)
    contracted over (k,c) = 47*256 = 12032 = 94 chunks of 128.
    bf16 is exact here: all values are integers <= 510 (even when > 256).
  - Host does only index re-encoding (one-hot/gather layout) + final
    alignment min-reduction; all q8 arithmetic happens on device.
"""

import numpy as np
import ml_dtypes

N_SRC = 4
M_TGT = 128
NCORES = 8
MLOC = M_TGT // NCORES      # 16 tgt rows per core
S = N_SRC * 100             # 400 src descriptors
TLOC = MLOC * 100           # 1600 tgt descriptors per core
NT = 13                     # t tiles of 128 (1600 -> padded 1664)
TPAD = NT * 128
K = 47                      # coefficient slots
NCH = (K * 256) // 128      # 94 contraction chunks
NCHP = 32                   # chunks per pass (3 passes, zero-padded to 96)
NPASS = 3

_CACHE = {}


def _build_nc():
    import concourse.bass as bass
    import concourse.mybir as mybir
    from concourse.tile import TileContext
    from concourse.tile_rust import add_dep_helper

    from concourse import tile as _tile_mod
    from concourse.vector_clock import ScopedClock as _ScopedClock

    if not getattr(_tile_mod.TileContext, "_drain_split_patched", False):
        def _drain_and_barrier(self, tick_clock, wait_clock):
            # walrus's setupSyncWait rejects instructions with many embedded
            # waits; spread the exit-drain's wait set over extra SP nops.
            drain_inst = self.nc.sync.drain()
            wait_clock.add_sem_waits(
                drain_inst.ins,
                _ScopedClock({None: tick_clock.global_clock}))
            si = drain_inst.ins.sync_info
            waits = list(si.on_wait or [])
            if len(waits) > 1:
                si.on_wait = waits[:1]
                for j in range(1, len(waits)):
                    nop = self.nc.sync.nop()
                    nop.ins.sync_info = mybir.SyncInfo(
                        on_wait=[waits[j]], on_update=[])
            self.nc.all_engine_barrier()
            assert self.sems is not None
            popped = self.nc._tile_sem_poison_stack.pop()
            assert popped is self._sem_poison
            self.nc.clear_and_free_semaphores(
                list(self.sems.allocated().values()))
            self.nc.all_engine_barrier()
        _tile_mod.TileContext._drain_and_barrier = _drain_and_barrier
        _tile_mod.TileContext._drain_split_patched = True

    nc = bass.Bass()
    rt_d = nc.dram_tensor("rt", [128, NCHP * S], mybir.dt.bfloat16,
                          kind="ExternalInput")
    b_d = nc.dram_tensor("b", [128, NT * NCHP * 128], mybir.dt.bfloat16,
                         kind="ExternalInput")
    d_d = nc.dram_tensor("d", [128, NT * S], mybir.dt.float32,
                         kind="ExternalOutput")

    with TileContext(nc) as tc:
        with (
            tc.tile_pool(name="rtp", bufs=1) as rtp,
            tc.tile_pool(name="bp", bufs=1) as bp,
            tc.tile_pool(name="psp", bufs=2, space=bass.MemorySpace.PSUM) as psp,
            tc.tile_pool(name="dp", bufs=1) as dp,
        ):
            rt_sb = rtp.tile([128, NCHP * S], mybir.dt.bfloat16)
            d_all = dp.tile([128, NT * S], mybir.dt.float32)
            # exactly 3 DMAs in the whole kernel: every tile is written
            # once (zero embedded waits on the descriptors) and the exit
            # drain only has to wait on a handful of procs.
            nc.sync.dma_start(rt_sb[:], rt_d[:])
            b_all = bp.tile([128, NT * NCHP * 128], mybir.dt.bfloat16)
            nc.sync.dma_start(b_all[:], b_d[:])
            for tt in range(NT):
                ps = psp.tile([128, S], mybir.dt.float32)
                for ch in range(NCHP):
                    nc.tensor.matmul(
                        ps[:],
                        b_all[:, (tt * NCHP + ch) * 128:
                              (tt * NCHP + ch + 1) * 128],
                        rt_sb[:, ch * S:(ch + 1) * S],
                        start=(ch == 0),
                        stop=(ch == NCHP - 1),
                    )
                nc.vector.tensor_copy(d_all[:, tt * S:(tt + 1) * S], ps[:])
            nc.sync.dma_start(d_d[:], d_all[:])
    return nc


def _get_nc():
    if "nc" not in _CACHE:
        _CACHE["nc"] = _build_nc()
    return _CACHE["nc"]


def _host_prep(q8_table, src_A, src_F, src_C, src_E):
    """Rt[(k,c), s] laid out as [128 part, 94 ch, 400 s] -> [128, 94*400]."""
    idxS = np.concatenate([
        src_A.reshape(S, 35),
        src_F.reshape(S, 10),
        src_C.reshape(S, 1),
        src_E.reshape(S, 1),
    ], axis=1)                                   # [400, 47]
    w = np.array([1.0] * 35 + [2.0] * 10 + [2.0, 1.0], np.float32)
    R = q8_table[idxS, :] * w[None, :, None]     # [400, 47, 256]
    Rt = R.transpose(1, 2, 0).reshape(K * 256, S)
    Rtp = np.zeros((NPASS * NCHP, 128, S), np.float32)
    Rtp[:NCH] = Rt.reshape(NCH, 128, S)
    return [np.ascontiguousarray(
        Rtp[p * NCHP:(p + 1) * NCHP].transpose(1, 0, 2).reshape(128, NCHP * S)
    ).astype(ml_dtypes.bfloat16) for p in range(NPASS)]


def _host_onehot(tgt_A, tgt_F, tgt_C, tgt_E, mlo, mhi):
    """B[(k,c), t] one-hot, laid out [13 tt, 128 part(c), 94*128 (ch,t)]."""
    nm = mhi - mlo
    t_cnt = nm * 100
    idxT = np.concatenate([
        tgt_A[mlo:mhi].reshape(t_cnt, 35),
        tgt_F[mlo:mhi].reshape(t_cnt, 10),
        tgt_C[mlo:mhi].reshape(t_cnt, 1),
        tgt_E[mlo:mhi].reshape(t_cnt, 1),
    ], axis=1)                                   # [1600, 47]
    B = np.zeros((K * 256, TPAD), np.float32)
    rows = (np.arange(K)[None, :] * 256 + idxT).ravel()
    cols = np.repeat(np.arange(t_cnt), K)
    B[rows, cols] = 1.0
    Bp = np.zeros((NPASS * NCHP, 128, NT, 128), np.float32)
    Bp[:NCH] = B.reshape(NCH, 128, NT, 128)
    return [np.ascontiguousarray(
        Bp[p * NCHP:(p + 1) * NCHP].transpose(1, 2, 0, 3)
        .reshape(128, NT * NCHP * 128)
    ).astype(ml_dtypes.bfloat16) for p in range(NPASS)]


def _reduce(D_full, align_10):
    """D_full: [128 m, 10 tc, 10 ta, 4 n, 10 sc, 10 sa] -> out [4, 128]."""
    cost = D_full.transpose(3, 0, 1, 4, 2, 5)    # [n,m,tc,sc,ta,sa]
    al = align_10[:, :10]                        # [60, 10]
    aligned = cost[..., al, np.arange(10)]       # [n,m,tc,sc,60,10]
    sum_diag = aligned.sum(-1)                   # [n,m,tc,sc,60]
    return sum_diag.reshape(N_SRC, M_TGT, -1).min(-1).astype(np.float32)


def kernel(q8_table, align_10,
           src_ArtCoeff, src_FdCoeff_q8, src_CirCoeff_q8, src_EccCoeff_q8,
           tgt_ArtCoeff, tgt_FdCoeff_q8, tgt_CirCoeff_q8, tgt_EccCoeff_q8,
           _trace=False):
    from concourse.bass_utils import run_bass_kernel_spmd

    q8 = np.asarray(q8_table, np.float32)
    rt_host = _host_prep(q8, src_ArtCoeff, src_FdCoeff_q8,
                         src_CirCoeff_q8, src_EccCoeff_q8)
    in_maps = []
    for i in range(NCORES):
        b_host = _host_onehot(tgt_ArtCoeff, tgt_FdCoeff_q8,
                              tgt_CirCoeff_q8, tgt_EccCoeff_q8,
                              i * MLOC, (i + 1) * MLOC)
        in_maps.append({"rt": rt_host, "b": b_host})

    nc = _get_nc()
    D_sum = [np.zeros((128, NT * S), np.float32) for _ in range(NCORES)]
    total_ns = 0
    for p in range(NPASS):
        maps_p = [{"rt": in_maps[i]["rt"][p], "b": in_maps[i]["b"][p]}
                  for i in range(NCORES)]
        res = run_bass_kernel_spmd(nc, maps_p, core_ids=list(range(NCORES)),
                                   trace=_trace)
        if res.exec_time_ns is not None:
            total_ns += res.exec_time_ns
        for i in range(NCORES):
            D_sum[i] += np.asarray(res.results[i]["d"], np.float32)
    _CACHE["last_result"] = res
    _CACHE["total_ns"] = total_ns if _trace else None

    # gather: per core D [13,128,400] -> [1664,400] -> [1600,400]
    D_parts = []
    for i in range(NCORES):
        d = D_sum[i].reshape(128, NT, S)
        d = d.transpose(1, 0, 2).reshape(TPAD, S)[:TLOC]
        D_parts.append(d.reshape(MLOC, 10, 10, N_SRC, 10, 10))
    D_full = np.concatenate(D_parts, axis=0)     # [128,10,10,4,10,10]
    return _reduce(D_full, np.asarray(align_10))



# revision 14
# speedup vs baseline: 3.2076x; 3.2076x over previous
"""LFD all-pairs distance kernel for 8 Trainium2 NeuronCores.

Strategy (data-parallel over tgt batch axis m, per sharding hint):
  - Each of the 8 cores owns 16 of the 128 tgt rows (1600 tgt descriptors).
  - The pairwise cost D[t, s] = sum_k w_k * q8_table[idxS[s,k], idxT[t,k]]
    (s = 400 src descriptors (n,sc,sa), t = 1600 tgt descriptors,
     k = 47 coefficient slots: 35 art + 10 fd(w=2) + cir(w=2) + ecc)
    is a one-hot contraction over (k, c):
        D[t_tile, s] = B^T @ Rt
    where Rt[(k,c), s] = q8_table[idxS[s,k], c]   (row-gathered table)
          B[(k,c), t]  = w_k if idxT[t,k] == c     (weighted one-hot)
  - Key optimization vs the dense scheme: per 128-target tile only the
    (k,c) rows actually used by some t in the tile enter the contraction
    (~4.6k of 12032, i.e. ~37 chunks of 128 instead of 94). The per-tile
    row sets are computed on host; Rt ships per-tile as uint8 (exact,
    half of bf16) and is converted to bf16 on DVE/ACT on-device,
    overlapped with the TensorE matmuls; B ships as fp8e4 (exact for
    {0,1,2}).  Single pass, per-tile double-buffered DMA.
  - Host does only index re-encoding (one-hot/gather layout) + final
    alignment min-reduction; all q8 arithmetic happens on device.
"""

import numpy as np
import ml_dtypes

N_SRC = 4
M_TGT = 128
NCORES = 8
MLOC = M_TGT // NCORES      # 16 tgt rows per core
S = N_SRC * 100             # 400 src descriptors
TLOC = MLOC * 100           # 1600 tgt descriptors per core
TILE_T = 128
NT = (TLOC + TILE_T - 1) // TILE_T   # 13 t tiles (last has 64 real t's)
K = 47                      # coefficient slots
W_K = np.array([1.0] * 35 + [2.0] * 10 + [2.0, 1.0], np.float32)

_CACHE = {}


def _install_tile_patch():
    import concourse.mybir as mybir
    from concourse import tile as _tile_mod
    from concourse.vector_clock import ScopedClock as _ScopedClock

    if getattr(_tile_mod.TileContext, "_drain_split_patched", False):
        return

    def _drain_and_barrier(self, tick_clock, wait_clock):
        # walrus's setupSyncWait rejects instructions with many embedded
        # waits; spread the exit-drain's wait set over extra SP nops.
        drain_inst = self.nc.sync.drain()
        wait_clock.add_sem_waits(
            drain_inst.ins,
            _ScopedClock({None: tick_clock.global_clock}))
        si = drain_inst.ins.sync_info
        waits = list(si.on_wait or [])
        if len(waits) > 1:
            si.on_wait = waits[:1]
            for j in range(1, len(waits)):
                nop = self.nc.sync.nop()
                nop.ins.sync_info = mybir.SyncInfo(
                    on_wait=[waits[j]], on_update=[])
        self.nc.all_engine_barrier()
        assert self.sems is not None
        popped = self.nc._tile_sem_poison_stack.pop()
        assert popped is self._sem_poison
        self.nc.clear_and_free_semaphores(
            list(self.sems.allocated().values()))
        self.nc.all_engine_barrier()

    _tile_mod.TileContext._drain_and_barrier = _drain_and_barrier
    _tile_mod.TileContext._drain_split_patched = True


def _build_nc(nch_list):
    import concourse.bass as bass
    import concourse.mybir as mybir
    from concourse.tile import TileContext

    _install_tile_patch()

    nch_tot = sum(nch_list)
    nch_max = max(nch_list)
    # walrus rejects DMA instructions with >1 semaphore wait. Each DMA'd
    # buffer has exactly one consumer engine; on buffer reuse the tile
    # framework emits [consumer-engine WAR wait, DMA-completion WAW wait].
    # The WAW wait is redundant there (the consumer's read already waited
    # on the previous DMA's completion), so _strip_waw_waits drops it.
    ha = max(1, nch_max * 6 // 10)               # DVE-converted chunks
    nc = bass.Bass()
    rta_d = nc.dram_tensor("rta", [128, nch_tot * S], mybir.dt.uint8,
                           kind="ExternalInput")
    b_d = nc.dram_tensor("b", [128, nch_tot * TILE_T], mybir.dt.float8e4,
                         kind="ExternalInput")
    d_d = nc.dram_tensor("d", [128, NT * S], mybir.dt.float32,
                         kind="ExternalOutput")

    with TileContext(nc) as tc:
        with (
            tc.tile_pool(name="rtua", bufs=3) as rtua_p,
            tc.tile_pool(name="rtub", bufs=3) as rtub_p,
            tc.tile_pool(name="rtba", bufs=3) as rtba_p,
            tc.tile_pool(name="rtbb", bufs=3) as rtbb_p,
            tc.tile_pool(name="bp", bufs=3) as b_p,
            tc.tile_pool(name="psp", bufs=4, space=bass.MemorySpace.PSUM) as ps_p,
            tc.tile_pool(name="dlo", bufs=NT) as dlo_p,
            tc.tile_pool(name="junk", bufs=1) as junk_p,
        ):
            junk = junk_p.tile([1, 4], mybir.dt.uint8)
            nc.gpsimd.memset(junk[:], 0)
            off = 0
            for tt, nch in enumerate(nch_list):
                na = min(ha, nch)
                nb = nch - na
                rtua = rtua_p.tile([128, ha * S], mybir.dt.uint8)
                nc.sync.dma_start(rtua[:, :na * S],
                                  rta_d[:, off * S:(off + na) * S])
                if nb:
                    rtub = rtub_p.tile([128, (nch_max - ha) * S],
                                       mybir.dt.uint8)
                    nc.sync.dma_start(rtub[:, :nb * S],
                                      rta_d[:, (off + na) * S:(off + nch) * S])
                bsb = b_p.tile([128, nch_max * TILE_T], mybir.dt.float8e4)
                nc.sync.dma_start(bsb[:, :nch * TILE_T],
                                  b_d[:, off * TILE_T:(off + nch) * TILE_T])
                # uint8 -> bf16 conversion split across DVE and ACT
                rtba = rtba_p.tile([128, ha * S], mybir.dt.bfloat16)
                nc.vector.tensor_copy(rtba[:, :na * S], rtua[:, :na * S])
                if nb:
                    rtbb = rtbb_p.tile([128, (nch_max - ha) * S],
                                       mybir.dt.bfloat16)
                    # dep-launder: walrus allows one wait per ACT
                    # instruction, but this convert needs two (WAR vs the
                    # matmuls reading the previous occupant + RAW on its
                    # input DMA). A tiny write into the tile's tail (read
                    # by the previous occupant's last matmul) carries the
                    # WAR alone; the convert's WAR then elides and it
                    # keeps only the DMA wait.
                    nc.scalar.copy(rtbb[0:1, nb * S - 4:nb * S],
                                   junk[0:1, 0:4])
                    nc.scalar.copy(rtbb[:, :nb * S], rtub[:, :nb * S])
                ps = ps_p.tile([128, S], mybir.dt.float32)
                for ch in range(nch):
                    src = (rtba[:, ch * S:(ch + 1) * S] if ch < na
                           else rtbb[:, (ch - na) * S:(ch - na + 1) * S])
                    nc.tensor.matmul(
                        ps[:],
                        bsb[:, ch * TILE_T:(ch + 1) * TILE_T],
                        src,
                        start=(ch == 0),
                        stop=(ch == nch - 1),
                    )
                # PSUM drain on DVE only (concurrent DVE+ACT readers of one
                # PSUM bank get serialized with an extra wait walrus can't
                # encode); write-once staging, single-wait output DMA.
                d_lo = dlo_p.tile([128, S], mybir.dt.float32)
                nc.vector.tensor_copy(d_lo[:], ps[:])
                nc.sync.dma_start(d_d[:, tt * S:(tt + 1) * S], d_lo[:])
                off += nch
    _strip_waw_waits(nc)
    return nc


_ENGINE_SEM_PREFIX = {
    "PE": "PE_",
    "DVE": "DVE_",
    "Activation": "Activation_",
    "SP": "SP_",
    "Pool": "Pool_",
}


def _strip_waw_waits(nc):
    """Reduce embedded sem waits to what walrus accepts (one per
    instruction for DMA/DVE/ACT). Two provably-redundant classes are
    dropped:
      - same-engine waits: engines execute their stream in order, so a
        wait on the instruction's own engine semaphore is already
        satisfied by program order;
      - DMA-completion (WAW) waits on reuse DMAs that also carry the
        consumer-engine WAR wait: the consumer's read of the old contents
        already waited on the old DMA's completion."""
    for inst in nc.all_instructions():
        si = getattr(inst, "sync_info", None)
        if not si or not si.on_wait or len(si.on_wait) <= 1:
            continue
        eng_name = getattr(getattr(inst, "engine", None), "name", "")
        own = _ENGINE_SEM_PREFIX.get(eng_name)
        waits = list(si.on_wait)
        if own is not None:
            waits = [w for w in waits if not (w.ant_name or "").startswith(own)]
        if type(inst).__name__ == "InstDMACopy" and len(waits) > 1:
            eng = [w for w in waits if "DMA" not in (w.ant_name or "")]
            assert len(eng) == 1, (
                f"unexpected DMA wait set on {inst.name}: "
                f"{[w.ant_name for w in si.on_wait]}"
            )
            waits = eng
        si.on_wait = waits


def _get_nc(nch_list):
    key = ("nc", tuple(nch_list))
    if key not in _CACHE:
        _CACHE[key] = _build_nc(nch_list)
    return _CACHE[key]


def _idx_concat(A, F, C, E, lo, hi, n_desc):
    return np.concatenate([
        A[lo:hi].reshape(n_desc, 35),
        F[lo:hi].reshape(n_desc, 10),
        C[lo:hi].reshape(n_desc, 1),
        E[lo:hi].reshape(n_desc, 1),
    ], axis=1).astype(np.int64)                  # [n_desc, 47]


def _host_prep(q8u8, idxS, idxT_cores):
    """Per-(core, tile) compressed row sets; returns per-core rt/b arrays
    plus the shared nch per tile position."""
    karr = np.arange(K, dtype=np.int64)[None, :] * 256
    # per (core, tile): sorted unique (k*256+c) rows
    rows_ct = []
    for idxT in idxT_cores:
        rows_t = []
        for tt in range(NT):
            sl = idxT[tt * TILE_T:min((tt + 1) * TILE_T, TLOC)]
            rows_t.append(np.unique((karr + sl).ravel()))
        rows_ct.append(rows_t)
    nch_list = [
        max((len(rows_ct[c][tt]) + 127) // 128 for c in range(NCORES))
        for tt in range(NT)
    ]
    rt_maps, b_maps = [], []
    for c, idxT in enumerate(idxT_cores):
        rt_parts, b_parts = [], []
        for tt in range(NT):
            nch = nch_list[tt]
            nrp = nch * 128
            rows = rows_ct[c][tt]
            nr = len(rows)
            rk = rows >> 8
            rc = rows & 255
            # Rt_tile [nrp, 400] uint8 = q8[idxS[s, rk], rc]
            rt = np.zeros((nrp, S), np.uint8)
            rt[:nr] = q8u8[idxS[:, rk], rc[None, :]].T
            # B [nrp, 128] = w_k one-hot
            sl = idxT[tt * TILE_T:min((tt + 1) * TILE_T, TLOC)]
            n_t = len(sl)
            pair = (karr + sl)                   # [n_t, 47]
            j = np.searchsorted(rows, pair.ravel())
            tcol = np.repeat(np.arange(n_t), K)
            bm = np.zeros((nrp, TILE_T), np.float32)
            bm[j, tcol] = np.tile(W_K, n_t)
            # SBUF layout [128 part, nch, S]
            rt_parts.append(np.ascontiguousarray(
                rt.reshape(nch, 128, S).transpose(1, 0, 2)))
            b_parts.append(np.ascontiguousarray(
                bm.reshape(nch, 128, TILE_T).transpose(1, 0, 2)))
        rt_maps.append(np.concatenate(
            [p.reshape(128, -1) for p in rt_parts], axis=1))
        b_maps.append(np.concatenate(
            [p.reshape(128, -1) for p in b_parts],
            axis=1).astype(ml_dtypes.float8_e4m3))
    return nch_list, rt_maps, b_maps


def _reduce(D_full, align_10):
    """D_full: [128 m, 10 tc, 10 ta, 4 n, 10 sc, 10 sa] -> out [4, 128]."""
    cost = D_full.transpose(3, 0, 1, 4, 2, 5)    # [n,m,tc,sc,ta,sa]
    al = align_10[:, :10]                        # [60, 10]
    aligned = cost[..., al, np.arange(10)]       # [n,m,tc,sc,60,10]
    sum_diag = aligned.sum(-1)                   # [n,m,tc,sc,60]
    return sum_diag.reshape(N_SRC, M_TGT, -1).min(-1).astype(np.float32)


def kernel(q8_table, align_10,
           src_ArtCoeff, src_FdCoeff_q8, src_CirCoeff_q8, src_EccCoeff_q8,
           tgt_ArtCoeff, tgt_FdCoeff_q8, tgt_CirCoeff_q8, tgt_EccCoeff_q8,
           _trace=False):
    from concourse.bass_utils import run_bass_kernel_spmd

    q8u8 = np.asarray(q8_table).astype(np.uint8)
    idxS = _idx_concat(np.asarray(src_ArtCoeff), np.asarray(src_FdCoeff_q8),
                       np.asarray(src_CirCoeff_q8), np.asarray(src_EccCoeff_q8),
                       0, N_SRC, S)
    tA = np.asarray(tgt_ArtCoeff)
    tF = np.asarray(tgt_FdCoeff_q8)
    tC = np.asarray(tgt_CirCoeff_q8)
    tE = np.asarray(tgt_EccCoeff_q8)
    idxT_cores = [
        _idx_concat(tA, tF, tC, tE, i * MLOC, (i + 1) * MLOC, TLOC)
        for i in range(NCORES)
    ]
    nch_list, rt_maps, b_maps = _host_prep(q8u8, idxS, idxT_cores)

    nc = _get_nc(nch_list)
    in_maps = [{"rta": rt_maps[i], "b": b_maps[i]} for i in range(NCORES)]
    res = run_bass_kernel_spmd(nc, in_maps, core_ids=list(range(NCORES)),
                               trace=_trace)
    _CACHE["last_result"] = res
    _CACHE["total_ns"] = res.exec_time_ns if _trace else None

    # gather: per core D [13,128,400] -> [1664,400] -> [1600,400]
    D_parts = []
    for i in range(NCORES):
        d = np.asarray(res.results[i]["d"], np.float32).reshape(128, NT, S)
        d = d.transpose(1, 0, 2).reshape(NT * TILE_T, S)[:TLOC]
        D_parts.append(d.reshape(MLOC, 10, 10, N_SRC, 10, 10))
    D_full = np.concatenate(D_parts, axis=0)     # [128,10,10,4,10,10]
    return _reduce(D_full, np.asarray(align_10))
